# revision 1
# baseline (speedup 1.0000x reference)
"""Trainium2 Bass kernel for dual-softmax mutual-NN feature matching (nn_Match).

Reference computation per batch n (l=4096, c=256):
    x   = (f1 @ f2^T) / 0.1                       [l, l]
    m   = softmax(x, axis=0) * softmax(x, axis=1)
    mutual-NN + threshold mask, gather-subtract, emit [c, h, w].

Distribution: 8 cores = 4 batches x 2 row-halves (2048 rows each).
Log-space decisions with VA := 20x - Q (Q = col LSE of 10x, P = row LSE):
    log m_ls = VA_ls - P_l
    j*_l     = argmax_s VA_ls          (max_index over materialised VA)
    T*_l     = VA*_l - P_l             (= log m at (l, j*))
    u2_ls    = VA_ls - P_l ; colW_s = max_l u2_ls   (partition all-reduce)
    mutual   = T* >= colW[j*] - eps    (Q cancels on both sides)
    matched  = mutual & (T* > ln 0.2)
Matmuls run as fp32->fp16 hi/lo split (3 fp16-rate matmuls per operand pair),
which keeps fp32-level precision of x; fp32r would be 3x cheaper but its
bf16-decomposition noise (~1e-2 abs) flips enough threshold decisions to fail
the 2e-2 gate (measured).  Pass R drains psum to SBUF via the Act engine so
the row max / VA materialisation / argmax all run from contiguous SBUF, the
row LSE needs a single shift (no per-chunk flash correction), and the colW
partials reuse the same noisy x as T* so the mutual compare cancels exactly.
A post-compile pass rewrites all activation-table loads to the combined
exp+ln set so per-tile Ln costs no table reloads.
"""

import os
import sys

import numpy as np

for _p in ("/opt/trn_rl_repo", "/root/.axon_site/_ro/trn_rl_repo"):
    if os.path.isdir(_p) and _p not in sys.path:
        sys.path.append(_p)

import concourse.bacc as bacc
import concourse.bass as bass
import concourse.bass_isa as bass_isa
import concourse.mybir as mybir
import concourse.tile as tile
from concourse.bass_utils import run_bass_kernel_spmd
from concourse.masks import make_identity

P = 128
F32 = mybir.dt.float32
F16 = mybir.dt.float16
U32 = mybir.dt.uint32
AX = mybir.AxisListType
OP = mybir.AluOpType
AF = mybir.ActivationFunctionType

NEG_BIG = -3.0e38
EPS_MUTUAL = 1.2e-3
LN_NUM = float(np.log(np.float32(0.2)))
ITEMP = 10.0  # 1 / TEMP

# act_info.json set 6 = natural_log_exp_and_others: exp, ln, copy, identity
ACT_SET_EXP_LN = 6


def _prep_matrix(nc, pools, src_dram, rows, c, dst_hi, dst_lo, idf16):
    """fp32 [rows, c] -> fp16 hi/lo, transposed into dst_{hi,lo} [P, c//P, rows]."""
    ct = c // P
    strip = 1024
    tps = strip // c
    n_strips = rows * c // (P * strip)
    src3 = src_dram.ap().rearrange("(t p) c -> p t c", p=P)
    for si in range(n_strips):
        nat = pools["prep_nat"].tile([P, strip], F32, tag="prep_nat")
        nc.sync.dma_start(nat[:], src3[:, si * tps : (si + 1) * tps, :])
        hi = pools["prep_hi"].tile([P, strip], F16, tag="prep_hi")
        lo = pools["prep_lo"].tile([P, strip], F16, tag="prep_lo")
        nc.vector.tensor_copy(hi[:], nat[:])
        nc.vector.tensor_tensor(out=lo[:], in0=nat[:], in1=hi[:], op=OP.subtract)
        for srcstrip, dst in ((hi, dst_hi), (lo, dst_lo)):
            for ci in range(ct):
                ps = pools["psum"].tile(
                    [P, tps * P], F16, tag="ps_sm", name="ps_tr", bufs=1
                )
                for k in range(tps):
                    nc.tensor.transpose(
                        out=ps[:, bass.ts(k, P)],
                        in_=srcstrip[:, k * c + ci * P : k * c + (ci + 1) * P],
                        identity=idf16[:],
                    )
                nc.scalar.copy(
                    out=dst[:, ci, si * tps * P : (si + 1) * tps * P], in_=ps[:]
                )


def emit_core_program(nc, cfg):
    lf, lr, c, chunk = cfg["lf"], cfg["lr"], cfg["c"], cfg["chunk"]
    nt = lr // P
    ct = c // P
    nch = lf // chunk
    nsub = chunk // 512

    f1r = nc.dram_tensor("f1r", [lr, c], F32, kind="ExternalInput")
    f1f = nc.dram_tensor("f1f", [lf, c], F32, kind="ExternalInput")
    f2f = nc.dram_tensor("f2f", [lf, c], F32, kind="ExternalInput")
    f2r = nc.dram_tensor("f2r", [lr, c], F32, kind="ExternalInput")
    out = nc.dram_tensor("out", [c, lr], F32, kind="ExternalOutput")

    q_own = nc.dram_tensor("q_own", [lr, 1], F32)
    q_full = nc.dram_tensor("q_full", [lf, 1], F32)
    cu_own = nc.dram_tensor("cu_own", [lf, 1], F32)
    cu_full = nc.dram_tensor("cu_full", [lf, 1], F32)

    groups = cfg["groups"]
    pair = len(groups[0]) > 1

    with tile.TileContext(nc) as tc:
        import contextlib

        with contextlib.ExitStack() as ctx:
            pools = {}

            def pool(name, bufs, space="SBUF"):
                pools[name] = ctx.enter_context(
                    tc.tile_pool(name=name, bufs=bufs, space=space)
                )
                return pools[name]

            pool("psum", 2, space="PSUM")
            pool("const", 1)
            pool("prep_nat", 2)
            pool("prep_hi", 2)
            pool("prep_lo", 2)
            pool("f1fTh", 1)
            pool("f1fTl", 1)
            pool("f2fTh", 1)
            pool("f2fTl", 1)
            pool("f1rTh", 1)
            pool("f1rTl", 1)
            pool("f2rTh", 1)
            pool("f2rTl", 1)
            pool("bcast", 1)
            pool("VA", 1)
            pool("xsb", 1)
            pool("rows16", 1)
            pool("stats", 1)
            pool("tiny", 8)
            pool("gstage", 2)
            pool("res", 2)
            pool("f2t", 2)
            pool("f1rt", 2)

            idf16 = pools["const"].tile([P, P], F16, tag="idf16")
            make_identity(nc, idf16[:])
            idf32 = pools["const"].tile([P, P], F32, tag="idf32")
            make_identity(nc, idf32[:])

            st = pools["stats"]
            q_sb = st.tile([P, nt], F32, tag="q_sb")
            pneg = st.tile([P, nt], F32, tag="pneg")
            va_star = st.tile([P, nt], F32, tag="va_star")
            jarr = st.tile([P, nt], U32, tag="jarr")

            # ---- operand prep (pass-Q operands first) ----
            f1fTh = pools["f1fTh"].tile([P, ct, lf], F16, tag="f1fTh")
            f1fTl = pools["f1fTl"].tile([P, ct, lf], F16, tag="f1fTl")
            f2fTh = pools["f2fTh"].tile([P, ct, lf], F16, tag="f2fTh")
            f2fTl = pools["f2fTl"].tile([P, ct, lf], F16, tag="f2fTl")
            f1rTh = pools["f1rTh"].tile([P, ct, lr], F16, tag="f1rTh")
            f1rTl = pools["f1rTl"].tile([P, ct, lr], F16, tag="f1rTl")
            f2rTh = pools["f2rTh"].tile([P, ct, lr], F16, tag="f2rTh")
            f2rTl = pools["f2rTl"].tile([P, ct, lr], F16, tag="f2rTl")
            _prep_matrix(nc, pools, f2r, lr, c, f2rTh, f2rTl, idf16)
            _prep_matrix(nc, pools, f1f, lf, c, f1fTh, f1fTl, idf16)

            def mm_tile(ps_list, l_hi, l_lo, r_hi, r_lo, t):
                for k in range(nch):
                    for ns in range(nsub):
                        s0 = k * chunk + ns * 512
                        pslice = ps_list[k][:, bass.ts(ns, 512)]
                        ops = []
                        for ci in range(ct):
                            wsl = bass.ds(t * P, P)
                            fsl = bass.ds(s0, 512)
                            ops.append((l_hi[:, ci, wsl], r_hi[:, ci, fsl]))
                            ops.append((l_hi[:, ci, wsl], r_lo[:, ci, fsl]))
                            ops.append((l_lo[:, ci, wsl], r_hi[:, ci, fsl]))
                        for i, (lw, rv) in enumerate(ops):
                            nc.tensor.matmul(
                                pslice,
                                lhsT=lw,
                                rhs=rv,
                                start=(i == 0),
                                stop=(i == len(ops) - 1),
                            )

            # ---- pass Q: x^T rows (own s) -> column LSE of x ----
            for t in range(nt):
                ps_list = [
                    pools["psum"].tile(
                        [P, chunk], F32, tag="ps_mm", name="ps_mm", bufs=3
                    )
                    for _ in range(nch)
                ]
                mm_tile(ps_list, f2rTh, f2rTl, f1fTh, f1fTl, t)
                tn = pools["tiny"]
                cm = tn.tile([P, nch], F32, tag="cm")
                es = tn.tile([P, nch], F32, tag="es")
                for k in range(nch):
                    nc.vector.reduce_max(cm[:, k : k + 1], ps_list[k][:], axis=AX.X)
                    negk = tn.tile([P, 1], F32, tag="negk")
                    nc.scalar.activation(
                        out=negk[:], in_=cm[:, k : k + 1], func=AF.Copy, scale=-ITEMP
                    )
                    nc.scalar.activation(
                        out=ps_list[k][:],
                        in_=ps_list[k][:],
                        func=AF.Exp,
                        bias=negk[:],
                        scale=ITEMP,
                        accum_out=es[:, k : k + 1],
                    )
                run = tn.tile([P, 1], F32, tag="run")
                nc.vector.reduce_max(run[:], cm[:], axis=AX.X)
                negrm = tn.tile([P, 1], F32, tag="negk", name="negrm")
                nc.scalar.activation(out=negrm[:], in_=run[:], func=AF.Copy, scale=-ITEMP)
                f4 = tn.tile([P, nch], F32, tag="f4")
                nc.scalar.activation(
                    out=f4[:], in_=cm[:], func=AF.Exp, bias=negrm[:], scale=ITEMP
                )
                ef = tn.tile([P, nch], F32, tag="f4", name="ef")
                nc.vector.tensor_tensor(out=ef[:], in0=es[:], in1=f4[:], op=OP.mult)
                acc = tn.tile([P, 1], F32, tag="acc")
                nc.vector.reduce_sum(acc[:], ef[:], axis=AX.X)
                lncs = tn.tile([P, 1], F32, tag="lncs")
                nc.scalar.activation(out=lncs[:], in_=acc[:], func=AF.Ln)
                nc.vector.scalar_tensor_tensor(
                    out=q_sb[:, t : t + 1],
                    in0=run[:],
                    scalar=ITEMP,
                    in1=lncs[:],
                    op0=OP.mult,
                    op1=OP.add,
                )

            # pass-R operand prep overlaps the tail of pass Q
            _prep_matrix(nc, pools, f1r, lr, c, f1rTh, f1rTl, idf16)
            _prep_matrix(nc, pools, f2f, lf, c, f2fTh, f2fTl, idf16)

            # ---- exchange Q halves (raw), broadcast ----
            nc.sync.dma_start(
                out=q_own.ap().rearrange("(t p) one -> p t one", p=P), in_=q_sb[:]
            )
            if not pair:
                for h0 in range(0, lf, lr):
                    nc.sync.dma_start(out=q_full[h0 : h0 + lr, :], in_=q_own.ap())
            else:
                nc.gpsimd.collective_compute(
                    "AllGather",
                    OP.bypass,
                    ins=[q_own.ap().opt()],
                    outs=[q_full.ap().opt()],
                    replica_groups=groups,
                )
            qstage = pools["VA"].tile([P, lf], F32, tag="VA", name="qstage")
            nc.sync.dma_start(
                out=qstage[0:1, :], in_=q_full.ap().rearrange("l one -> one l")
            )
            q1b = pools["bcast"].tile([P, lf], F32, tag="b1b")
            nc.gpsimd.partition_broadcast(q1b[:], qstage[0:1, :])

            rows16 = pools["rows16"].tile([P, lf], F32, tag="rows16")
            nc.vector.memset(rows16[:], NEG_BIG)

            # ---- pass R: x rows (own l) ----
            for t in range(nt):
                ps_list = [
                    pools["psum"].tile(
                        [P, chunk], F32, tag="ps_mm", name="ps_mm", bufs=3
                    )
                    for _ in range(nch)
                ]
                mm_tile(ps_list, f1rTh, f1rTl, f2fTh, f2fTl, t)
                tn = pools["tiny"]
                # drain psum to contiguous SBUF via Act; per-chunk shifted
                # exp back into psum (chunk-local chains keep psum bufs free)
                xsb = pools["xsb"].tile([P, lf], F32, tag="xsb")
                cm = tn.tile([P, nch], F32, tag="cm")
                es = tn.tile([P, nch], F32, tag="es")
                for k in range(nch):
                    sl = bass.ts(k, chunk)
                    nc.scalar.copy(out=xsb[:, sl], in_=ps_list[k][:])
                    nc.vector.reduce_max(cm[:, k : k + 1], xsb[:, sl], axis=AX.X)
                    negk = tn.tile([P, 1], F32, tag="negk")
                    nc.scalar.activation(
                        out=negk[:], in_=cm[:, k : k + 1], func=AF.Copy, scale=-ITEMP
                    )
                    nc.scalar.activation(
                        out=ps_list[k][:],
                        in_=xsb[:, sl],
                        func=AF.Exp,
                        bias=negk[:],
                        scale=ITEMP,
                        accum_out=es[:, k : k + 1],
                    )
                # VA = 20x - Q (DVE, SBUF fast path)
                VA = pools["VA"].tile([P, lf], F32, tag="VA")
                nc.vector.scalar_tensor_tensor(
                    out=VA[:],
                    in0=xsb[:],
                    scalar=20.0,
                    in1=q1b[:],
                    op0=OP.mult,
                    op1=OP.subtract,
                )
                # flash combine: P = 10 run + ln(sum es_k exp(10(cm_k - run)))
                run = tn.tile([P, 1], F32, tag="run")
                nc.vector.reduce_max(run[:], cm[:], axis=AX.X)
                negrm = tn.tile([P, 1], F32, tag="negk", name="negrm")
                nc.scalar.activation(out=negrm[:], in_=run[:], func=AF.Copy, scale=-ITEMP)
                f4 = tn.tile([P, nch], F32, tag="f4")
                nc.scalar.activation(
                    out=f4[:], in_=cm[:], func=AF.Exp, bias=negrm[:], scale=ITEMP
                )
                ef = tn.tile([P, nch], F32, tag="f4", name="ef")
                nc.vector.tensor_tensor(out=ef[:], in0=es[:], in1=f4[:], op=OP.mult)
                acc = tn.tile([P, 1], F32, tag="acc")
                nc.vector.reduce_sum(acc[:], ef[:], axis=AX.X)
                lnacc = tn.tile([P, 1], F32, tag="lncs")
                nc.scalar.activation(out=lnacc[:], in_=acc[:], func=AF.Ln)
                # p_neg = -(10 run + ln acc)
                nc.vector.scalar_tensor_tensor(
                    out=pneg[:, t : t + 1],
                    in0=run[:],
                    scalar=-ITEMP,
                    in1=lnacc[:],
                    op0=OP.mult,
                    op1=OP.subtract,
                )
                # VA* and argmax
                nc.vector.reduce_max(va_star[:, t : t + 1], VA[:], axis=AX.X)
                mx8 = tn.tile([P, 8], F32, tag="mx8")
                nc.vector.tensor_copy(
                    mx8[:], va_star[:, t : t + 1].to_broadcast([P, 8])
                )
                idx8 = tn.tile([P, 8], U32, tag="idx8")
                nc.vector.max_index(idx8[:], mx8[:], VA[:])
                nc.vector.tensor_copy(jarr[:, t : t + 1], idx8[:, 0:1])
                # u2 = VA + p_neg into xsb (x dead there), colW partial
                nc.gpsimd.tensor_scalar(
                    out=xsb[:],
                    in0=VA[:],
                    scalar1=1.0,
                    scalar2=pneg[:, t : t + 1],
                    op0=OP.mult,
                    op1=OP.add,
                )
                nc.gpsimd.partition_all_reduce(
                    VA[:], xsb[:], channels=P, reduce_op=bass_isa.ReduceOp.max
                )
                nc.sync.dma_start(rows16[t : t + 1, :], VA[0:1, :])

            # ---- colW: combine partials, exchange ----
            parf = pools["xsb"].tile([P, lf], F32, tag="xsb", name="parf")
            nc.gpsimd.partition_all_reduce(
                parf[:], rows16[:], channels=P, reduce_op=bass_isa.ReduceOp.max
            )
            nc.sync.dma_start(
                out=cu_own.ap().rearrange("l one -> one l"), in_=parf[0:1, :]
            )
            if not pair:
                nc.sync.dma_start(out=cu_full.ap(), in_=cu_own.ap())
            else:
                nc.gpsimd.collective_compute(
                    "AllReduce",
                    OP.max,
                    ins=[cu_own.ap().opt()],
                    outs=[cu_full.ap().opt()],
                    replica_groups=groups,
                )

            # ---- tail ----
            f1r_tiled = f1r.ap().rearrange("(t p) c -> p t c", p=P)
            group = 2
            ps_out = []
            for t in range(nt):
                tn = pools["tiny"]
                cug = tn.tile([P, 1], F32, tag="cug")
                nc.gpsimd.indirect_dma_start(
                    out=cug[:],
                    out_offset=None,
                    in_=cu_full[:],
                    in_offset=bass.IndirectOffsetOnAxis(ap=jarr[:, t : t + 1], axis=0),
                )
                f2t = pools["f2t"].tile([P, c], F32, tag="f2t")
                nc.gpsimd.indirect_dma_start(
                    out=f2t[:],
                    out_offset=None,
                    in_=f2f[:],
                    in_offset=bass.IndirectOffsetOnAxis(ap=jarr[:, t : t + 1], axis=0),
                )
                tstar = tn.tile([P, 1], F32, tag="tstar")
                nc.vector.tensor_tensor(
                    out=tstar[:],
                    in0=va_star[:, t : t + 1],
                    in1=pneg[:, t : t + 1],
                    op=OP.add,
                )
                thr = tn.tile([P, 1], F32, tag="thr")
                nc.vector.tensor_scalar(
                    out=thr[:],
                    in0=tstar[:],
                    scalar1=LN_NUM,
                    scalar2=None,
                    op0=OP.is_gt,
                )
                mut = tn.tile([P, 1], F32, tag="mut")
                nc.vector.tensor_scalar(
                    out=mut[:],
                    in0=tstar[:],
                    scalar1=EPS_MUTUAL,
                    scalar2=cug[:],
                    op0=OP.add,
                    op1=OP.is_ge,
                )
                negmask = tn.tile([P, 1], F32, tag="negmask")
                nc.vector.scalar_tensor_tensor(
                    out=negmask[:],
                    in0=mut[:],
                    scalar=-1.0,
                    in1=thr[:],
                    op0=OP.mult,
                    op1=OP.mult,
                )
                f1t = pools["f1rt"].tile([P, c], F32, tag="f1rt")
                nc.sync.dma_start(f1t[:], f1r_tiled[:, t, :])
                res = pools["res"].tile([P, c], F32, tag="res")
                nc.vector.scalar_tensor_tensor(
                    out=res[:],
                    in0=f2t[:],
                    scalar=negmask[:],
                    in1=f1t[:],
                    op0=OP.mult,
                    op1=OP.add,
                )
                gi = t % group
                if gi == 0:
                    ps_out = pools["psum"].tile(
                        [P, ct * group * P], F32, tag="ps_out", name="ps_out", bufs=1
                    )
                for ci in range(ct):
                    nc.tensor.transpose(
                        out=ps_out[
                            :, ci * group * P + gi * P : ci * group * P + (gi + 1) * P
                        ],
                        in_=res[:, bass.ts(ci, P)],
                        identity=idf32[:],
                    )
                if gi == group - 1 or t == nt - 1:
                    g0 = (t // group) * group
                    gn = t - g0 + 1
                    gs = pools["gstage"].tile([P, ct, group * P], F32, tag="gstage")
                    for ci in range(ct):
                        nc.scalar.copy(
                            out=gs[:, ci, : gn * P],
                            in_=ps_out[:, ci * group * P : ci * group * P + gn * P],
                        )
                        nc.sync.dma_start(
                            out=out[ci * P : (ci + 1) * P, g0 * P : (g0 + gn) * P],
                            in_=gs[:, ci, : gn * P],
                        )
    return nc


_ENGINE_ATTR = {
    mybir.EngineType.SP: "sync",
    mybir.EngineType.Pool: "gpsimd",
    mybir.EngineType.DVE: "vector",
    mybir.EngineType.Activation: "scalar",
    mybir.EngineType.PE: "tensor",
}


def _make_nop(nc, engine_type):
    """Create a detached InstNoOp on the given engine."""
    eng = getattr(nc, _ENGINE_ATTR[engine_type])
    r = eng.nop(nofuse=True)
    target = r.ins if hasattr(r, "ins") else r
    for fn in nc.m.functions:
        for blk in fn.blocks:
            lst = blk.instructions
            if lst and lst[-1] is target:
                blk.instructions = lst[:-1]
                return target
    raise RuntimeError("freshly created nop not found")


def _fix_act_tables(nc):
    """Collapse activation-table loads to the combined exp+ln set.

    The table-load insertion pass greedily picks the first set containing
    each function, thrashing between the exp-only and ln-only sets (446 ns
    per reload).  Set 6 (natural_log_exp_and_others) contains exp, ln, copy
    and identity -- every function this kernel uses -- so one load suffices.
    Loads carrying sync_info are kept (retargeted to set 6); bare repeats
    are dropped.
    """
    n_seen = 0
    for fn in nc.m.functions:
        for blk in fn.blocks:
            new = []
            for inst in blk.instructions:
                if isinstance(inst, mybir.InstLoadActFuncSet):
                    si = getattr(inst, "sync_info", None)
                    has_sems = si is not None and (
                        len(si.on_wait) > 0 or len(si.on_update) > 0
                    )
                    if n_seen == 0 or has_sems:
                        inst.act_func_set_id = ACT_SET_EXP_LN
                        new.append(inst)
                    n_seen += 1
                    continue
                new.append(inst)
            blk.instructions = new
    return n_seen


_PROGRAM_CACHE = {}


def build_program(lf=4096, lr=2048, c=256, chunk=1024, n_cores=8):
    key = (lf, lr, c, chunk, n_cores)
    if key in _PROGRAM_CACHE:
        return _PROGRAM_CACHE[key]
    nc = bacc.Bacc(
        "TRN2",
        target_bir_lowering=False,
        debug=False,
        num_devices=n_cores,
    )
    if n_cores == 1:
        groups = [[0]]
    else:
        groups = [[i, i + 1] for i in range(0, n_cores, 2)]
    cfg = {"lf": lf, "lr": lr, "c": c, "chunk": chunk, "groups": groups}
    emit_core_program(nc, cfg)
    nc.compile()
    _fix_act_tables(nc)
    _PROGRAM_CACHE[key] = nc
    return nc


def make_in_maps(f1, f2, n_cores=8):
    bsz, l, cc = f1.shape
    halves = n_cores // bsz
    lr = l // halves
    in_maps = []
    for core in range(n_cores):
        n = core // halves
        q = core % halves
        in_maps.append(
            {
                "f1r": np.ascontiguousarray(f1[n, q * lr : (q + 1) * lr]),
                "f1f": np.ascontiguousarray(f1[n]),
                "f2f": np.ascontiguousarray(f2[n]),
                "f2r": np.ascontiguousarray(f2[n, q * lr : (q + 1) * lr]),
            }
        )
    return in_maps


def kernel(feature1, feature2, b=4, c=256, h=64, w=64, **_ignored):
    f1 = np.ascontiguousarray(np.asarray(feature1, dtype=np.float32))
    f2 = np.ascontiguousarray(np.asarray(feature2, dtype=np.float32))
    bsz, l, cc = f1.shape
    h = int(h) if np.ndim(h) == 0 else 64
    w = l // h
    n_cores = 8
    halves = n_cores // bsz
    lr = l // halves
    nc = build_program(lf=l, lr=lr, c=cc, chunk=1024, n_cores=n_cores)
    in_maps = make_in_maps(f1, f2, n_cores)
    results = run_bass_kernel_spmd(nc, in_maps, core_ids=list(range(n_cores)))
    hh = h // halves
    outp = np.empty((bsz, cc, h, w), dtype=np.float32)
    for core in range(n_cores):
        n = core // halves
        q = core % halves
        outp[n, :, q * hh : (q + 1) * hh, :] = results.results[core]["out"].reshape(
            cc, hh, w
        )
    return outp


if __name__ == "__main__":
    f1 = np.load("/root/problem/f1.npy")
    f2 = np.load("/root/problem/f2.npy")
    res = kernel(f1, f2)
    exp = np.load("/root/problem/expected.npy")
    err = np.linalg.norm(res - exp) / np.linalg.norm(exp)
    print("Relative error:", err)

